# revision 47
# baseline (speedup 1.0000x reference)
"""Trainium2 Bass kernel for nn_DQA_graph (GNN message passing, DQA attention).

Strategy (data-parallel over nodes, 8 cores):
  - Nodes padded to 50176 = 8 cores x 49 tiles x 128 rows; core c owns rows
    [c*6272, (c+1)*6272).
  - Node states live in a packed DRAM table whose rows hold [x | sb | sa]
    where sa/sb are the per-head DQA score contributions (h @ wa.T, h @ wb.T).
    The neighbor gather fetches x AND sb in one row read.
      * f32 mode  (DQA_BF16=0): rows of 192 f32 (768B): x f32[128] | sb | sa
      * bf16 mode (DQA_BF16=1): rows of 128 f32 (512B): x bf16[128] packed in
        64 f32 slots | sb f32[4] | sa f32[4] | pad
  - Step 0 reads a HOST-precomputed replicated table (no pack phase and no
    step-0 AllGather on device); each step s>=1 gathers from an AllGather of
    the packed rows produced by step s-1.
  - Per-node neighbor lists are pre-sorted ascending (host), so the K=32
    gather columns of a 128-node tile are order statistics; greedy grouping
    packs columns into windows whose index span fits dma_gather's int16
    range, with the window base supplied at runtime per (core, tile, window)
    from a metadata tensor (the program is SPMD-uniform across cores).
  - The gather is k-major: gathered row (k*128 + t) lands at partition t,
    chunk k -> the xg tile is directly [node t, slot k, row] with no
    transpose.
  - Scores/softmax run on ACT+DVE entirely in [t, *] layout; the weighted
    sum is 4 interleaved chains of scalar_tensor_tensor FMAs (per-partition
    scalar, fp16 accumulators) for DVE ILP.
  - Window base registers are bulk-loaded 8 at a time (double-banked) to
    amortize the ~0.8us/reg_load Pool-sequencer cost.
  - Perf notes (measured): the kernel is gather-bound; the dma_gather is
    HBM-latency-bound per descriptor (~140ns/row across 16 SDMA engines,
    ~53GB/s effective), so descriptor COUNT (one per neighbor row) is what
    matters, not row width. Runner pre-shards device inputs (NamedSharding)
    so each dispatch moves no input bytes.
"""
import os
import sys

sys.path.insert(0, "/opt/trn_rl_repo")
import numpy as np

import concourse.bacc as bacc
import concourse.bass as bass
import concourse.mybir as mybir
from concourse.bass_utils import run_bass_kernel_spmd
from concourse.tile import TileContext

# problem constants (hardcoded per harness contract)
N, K, S, H = 50000, 32, 128, 4
NCORES = 8
P = 128
TPC = 49                      # tiles per core
NPAD = NCORES * TPC * P       # 50176
SHARD = TPC * P               # 6272 rows per core
BF16 = bool(int(os.environ.get("DQA_BF16", "1")))
if BF16:
    RW = 128                  # packed row width (f32 slots) = 512B
    XW = 64                   # f32 slots holding the (bf16) x payload
else:
    RW = 192                  # 768B rows
    XW = 128
OFF_SB, OFF_SA = XW, XW + H
PACKW = XW + 2 * H            # meaningful prefix of a packed row
MAXW = 32768                  # int16 index window (rows)
MAXM = int(os.environ.get("DQA_MAXM", "16"))  # max columns per gather call
SINGLEPKT = MAXM <= 8
CHAIN16 = bool(int(os.environ.get("DQA_CHAIN16", "1")))  # fp16 FMA chains
NEG = -50.0
ALPHA = 0.01                  # leaky relu slope
FT = mybir.dt.float32
BT = mybir.dt.bfloat16


def _to_bf16_bits(a):
    """f32 ndarray -> uint16 bf16 bits (round to nearest even)."""
    v = a.astype(np.float32).view(np.uint32)
    r = (v + 0x7FFF + ((v >> 16) & 1)) >> 16
    return r.astype(np.uint16)


def _pack_rows(x, sb, sa):
    """Pack [n,S] f32 x (+[n,H] sb, sa) into [n, RW] f32-viewed rows."""
    n = x.shape[0]
    out = np.zeros((n, RW), np.float32)
    if BF16:
        bits = _to_bf16_bits(x)                      # [n, S] uint16
        out[:, :XW] = bits.view(np.uint32).view(np.float32)
    else:
        out[:, :XW] = x
    out[:, OFF_SB:OFF_SB + H] = sb
    out[:, OFF_SA:OFF_SA + H] = sa
    return out


# ----------------------------------------------------------------- host prep
def _prep_graph(neighbors, mask, maxm=None):
    """Valid-compacted, v-sorted gather plan.

    Nodes are permuted by valid-neighbor count v so that each tile position i
    draws its 8 cores' tiles from one contiguous v-sorted block of 1024 nodes
    -> every core shares the same per-tile column count K_i = max v in block
    (the SPMD program needs shared loop bounds). Each node's VALID neighbors
    are compacted to the front (sorted ascending for windowing); columns
    k >= v_t are padded with the node's last valid neighbor (masked -50).
    Invalid slots beyond K_i are simply never gathered (~25% fewer
    descriptors; the gather is HBM-latency-bound per descriptor).

    Returns dict with per-core input arrays, the window plan, per-tile K_i,
    and the node permutation (kernel() un-permutes the output)."""
    if maxm is None:
        maxm = MAXM
    nbr = np.asarray(neighbors, np.int64)
    msk = np.asarray(mask, bool)

    # padded node table: pads have v=0
    v = np.zeros(NPAD, np.int64)
    v[:N] = msk.sum(axis=1)
    perm = np.argsort(v, kind="stable")          # v-sorted rank s -> old node

    # table row id r (core-major: r = c*SHARD + i*P + t) for sorted rank s
    # with i = s // 1024, c = (s // 128) % 8, t = s % 128 — so tile position
    # i on every core draws from the same v-sorted block of 1024 nodes.
    s_arr = np.arange(NPAD)
    i_arr, c_arr, t_arr = s_arr // (NCORES * P), (s_arr // P) % NCORES, s_arr % P
    r_of_s = c_arr * SHARD + i_arr * P + t_arr
    old_of_row = np.empty(NPAD, np.int64)
    old_of_row[r_of_s] = perm                     # row r -> old node id
    row_of_old = np.empty(NPAD, np.int64)
    row_of_old[old_of_row] = np.arange(NPAD)      # old node id -> row r

    v_row = v[old_of_row]
    # per row: compacted, remapped, ascending-sorted valid neighbors
    nbr_c = np.zeros((NPAD, K), np.int64)
    # and full original-style sorted-32 lists (fallback mode)
    nbr_f = np.zeros((NPAD, K), np.int64)
    msk_f = np.zeros((NPAD, K), bool)
    for r in range(NPAD):
        o = old_of_row[r]
        if o < N:
            vals_all = row_of_old[nbr[o]]
            order = np.argsort(vals_all, kind="stable")
            nbr_f[r] = vals_all[order]
            msk_f[r] = msk[o][order]
            if v_row[r] > 0:
                vals = np.sort(vals_all[msk[o]])
                nbr_c[r, :v_row[r]] = vals
                nbr_c[r, v_row[r]:] = vals[-1]    # provisional pad
            else:
                nbr_c[r, :] = r
        else:
            nbr_c[r, :] = r                       # pad node (masked anyway)
            nbr_f[r, :] = r
    valid_col = np.arange(K)[None, :] < v_row[:, None]   # [NPAD, K]
    # pad slots are masked (-50) so their value only matters for window
    # spans: use the tile-column median of valid entries to keep columns
    # tight (last-valid padding puts a node's max in mid-quantile columns)
    for blk in range(NPAD // P):
        rows_b = slice(blk * P, (blk + 1) * P)
        for k in range(K):
            vc = valid_col[rows_b, k]
            if vc.all():
                continue
            if vc.any():
                med = int(np.median(nbr_c[rows_b, k][vc]))
            else:
                med = blk * P                     # all-pad column (v=0 tile)
            col = nbr_c[rows_b, k]
            col[~vc] = med
            nbr_c[rows_b, k] = col

    # per-tile-position column count, shared across cores (>=1)
    KT = [max(1, int(v[perm[i * NCORES * P:(i + 1) * NCORES * P]].max()))
          for i in range(TPC)]

    cols_c = nbr_c.reshape(NCORES, TPC, P, K).transpose(0, 1, 3, 2)
    cols_f = nbr_f.reshape(NCORES, TPC, P, K).transpose(0, 1, 3, 2)
    span_c = cols_c.max(axis=3) - cols_c.min(axis=3)  # [NCORES, TPC, K]

    # per tile position: compact unless any core has a single-column span
    # overflow within the compact K range -> fall back to the full-32 layout
    cols = cols_f.copy()
    maskneg = np.where(msk_f, 0.0, NEG).astype(np.float32)
    mask_rows = maskneg.reshape(NCORES, TPC, P, K)
    modes = []
    for i in range(TPC):
        if (span_c[:, i, :KT[i]] < MAXW).all():
            cols[:, i] = cols_c[:, i]
            mask_rows[:, i] = np.where(
                valid_col.reshape(NCORES, TPC, P, K)[:, i], 0.0, NEG)
            modes.append("compact")
        else:
            KT[i] = K
            modes.append("full")
    maskneg = mask_rows.reshape(NPAD, K).astype(np.float32)
    cmin = cols.min(axis=3)
    cmax = cols.max(axis=3)

    plan = []  # per tile position: list of (k0, k1), k1 <= KT[i]
    for i in range(TPC):
        wins = []
        k0 = 0
        while k0 < KT[i]:
            lo = cmin[:, i, k0].copy()
            hi = cmax[:, i, k0].copy()
            assert (hi - lo < MAXW).all(), "single column exceeds window"
            k1 = k0 + 1
            while k1 < KT[i] and k1 - k0 < maxm:
                nlo = np.minimum(lo, cmin[:, i, k1])
                nhi = np.maximum(hi, cmax[:, i, k1])
                if (nhi - nlo >= MAXW).any():
                    break
                lo, hi = nlo, nhi
                k1 += 1
            wins.append((k0, k1))
            k0 = k1
        plan.append(wins)

    # per-core blobs (variable total idx columns, shared shape across cores)
    idx_cols_total = sum((k1 - k0) * P for p in plan for (k0, k1) in p) // 16
    idx_blobs, meta_blobs = [], []
    for c in range(NCORES):
        idx_blob = np.zeros((16, idx_cols_total), np.int16)
        bases = []
        off = 0
        for i in range(TPC):
            tile_cols = cols[c, i]  # [K, P]
            for (k0, k1) in plan[i]:
                base = int(tile_cols[k0:k1].min())
                bases.append(base * RW)
                rel = (tile_cols[k0:k1] - base).astype(np.int64)  # [m, P]
                assert rel.min() >= 0 and rel.max() < MAXW
                flat = rel.reshape(-1).astype(np.int16)  # k-major
                m16 = flat.shape[0] // 16
                idx_blob[:, off:off + m16] = flat.reshape(m16, 16).T
                off += m16
        assert off == idx_cols_total
        idx_blobs.append(np.tile(idx_blob, (8, 1)))  # replicate to 128 parts
        meta_blobs.append(np.asarray(bases, np.int32).reshape(1, -1))

    return {
        "plan": plan,
        "kt": KT,
        "modes": modes,
        "idx_blobs": idx_blobs,
        "meta_blobs": meta_blobs,
        "maskneg": maskneg,          # already in row order
        "old_of_row": old_of_row,    # row r -> original node id
        "n_windows": len(meta_blobs[0][0]),
        "idx_cols_total": idx_cols_total,
    }


# ------------------------------------------------------------- device build
def _build(steps, plan, n_windows, tpc=TPC, mode="full", reps=1,
           chain16=None, singlepkt=None, gbufs=3, kt=None, idx_cols=None):
    """Build the SPMD Bacc module for `steps` propagation steps.

    mode: "full" | "gather" (skip per-tile compute) | "compute" (skip
    gathers, memset xg instead).
    reps: replicate the whole computation inside the NEFF (timing use:
    (wall_R - wall_1)/(R-1) cancels dispatch overhead)."""
    if chain16 is None:
        chain16 = CHAIN16
    if singlepkt is None:
        singlepkt = max(k1 - k0 for p in plan for (k0, k1) in p) <= 8
    if kt is None:
        kt = [K] * tpc
    if idx_cols is None:
        idx_cols = SHARD * K // 16
    nc = bacc.Bacc()
    t_tab0 = nc.dram_tensor("table0", [NPAD, RW], FT, kind="ExternalInput")
    t_own0 = nc.dram_tensor("own0", [SHARD, RW], FT, kind="ExternalInput")
    t_idx = nc.dram_tensor("idx_blob", [128, idx_cols], mybir.dt.int16,
                           kind="ExternalInput")
    t_meta = nc.dram_tensor("meta", [1, n_windows], mybir.dt.int32,
                            kind="ExternalInput")
    t_mn = nc.dram_tensor("maskneg", [SHARD, K], FT, kind="ExternalInput")
    t_wab = nc.dram_tensor("wab", [S, 2 * H], FT, kind="ExternalInput")
    t_bb = nc.dram_tensor("bb", [P, H], FT, kind="ExternalInput")
    t_ident = nc.dram_tensor("ident", [P, P], FT, kind="ExternalInput")
    t_out = nc.dram_tensor("out_shard", [SHARD, S], FT, kind="ExternalOutput")

    shards = [nc.dram_tensor(f"shard{s}", [SHARD, RW], FT)
              for s in range(1, steps)]
    tables = [nc.dram_tensor(f"table{s}", [NPAD, RW], FT, addr_space="Shared")
              for s in range(1, steps)]

    pool_regs = [list(nc.alloc_registers(f"gbase{j}",
                                         engines=[mybir.EngineType.Pool]))[0]
                 for j in range(16)]
    GRP = 8  # window bases bulk-loaded per reg_load, double-banked

    for rep in range(reps):
      for s in range(steps):
        if s > 0:
            # AllGather shard_{s} -> table_{s}
            with nc.Block() as block, \
                 nc.semaphore(f"ccs{rep}_{s}") as cc_sem:
                @block.gpsimd
                def _(gpsimd, s=s, cc_sem=cc_sem):
                    gpsimd.collective_compute(
                        "AllGather", mybir.AluOpType.bypass,
                        replica_groups=[list(range(NCORES))],
                        ins=[shards[s - 1][:]], outs=[tables[s - 1][:]],
                    ).then_inc(cc_sem, 1)
                    gpsimd.wait_ge(cc_sem, 1)

        last = (s == steps - 1)
        with TileContext(nc) as tc:
            with tc.tile_pool(name=f"g{rep}_{s}", bufs=gbufs) as gpool, \
                 tc.tile_pool(name=f"sm{rep}_{s}", bufs=3) as spool, \
                 tc.tile_pool(name=f"c{rep}_{s}", bufs=1) as cpool, \
                 tc.tile_pool(name=f"ps{rep}_{s}", bufs=2, space="PSUM") as pp:
                idxb = cpool.tile([128, idx_cols], mybir.dt.int16)
                meta = cpool.tile([1, n_windows], mybir.dt.int32)
                wab = cpool.tile([S, 2 * H], FT)
                bb = cpool.tile([P, H], FT)
                ident = cpool.tile([P, P], FT)
                nc.sync.dma_start(out=idxb[:], in_=t_idx[:])
                nc.sync.dma_start(out=meta[:], in_=t_meta[:])
                nc.sync.dma_start(out=wab[:], in_=t_wab[:])
                nc.sync.dma_start(out=bb[:], in_=t_bb[:])
                nc.sync.dma_start(out=ident[:], in_=t_ident[:])
                # bulk per-step loads: own rows + mask for all tiles
                own_src = t_own0 if s == 0 else shards[s - 1]
                own_all = cpool.tile([P, tpc, PACKW], FT)
                nc.sync.dma_start(
                    out=own_all[:],
                    in_=own_src[:tpc * P, :PACKW].rearrange(
                        "(i p) w -> p i w", p=P))
                mn_all = cpool.tile([P, tpc, K], FT)
                nc.sync.dma_start(
                    out=mn_all[:],
                    in_=t_mn[:tpc * P, :].rearrange("(i p) k -> p i k", p=P))

                src_rows = (t_tab0 if s == 0 else tables[s - 1])[:]
                widx = 0
                ioff = 0
                for i in range(tpc):
                    rows = slice(i * P, (i + 1) * P)
                    kti = kt[i]
                    if mode == "compute":
                        if i == 0:
                            xg_shared = cpool.tile([P, K * RW], FT,
                                                   name=f"xgsh{rep}_{s}")
                            nc.vector.memset(xg_shared[:], 0.01)
                        xg3 = xg_shared[:].rearrange("p (k w) -> p k w", w=RW)
                        widx += len(plan[i])
                        ioff += K * P // 16
                    elif mode == "gsep":
                        # timing probe: each window gathers into its own tile
                        if i == 0 and rep == 0 and s == 0:
                            pass
                        for (k0, k1) in plan[i]:
                            m = k1 - k0
                            xgw = gpool.tile([P, MAXM * RW], FT, tag="xgw")
                            reg = pool_regs[widx % len(pool_regs)]
                            nc.reg_load(reg, meta[0:1, widx:widx + 1])
                            src = bass.AP(src_rows.tensor,
                                          bass.RuntimeValue(reg), src_rows.ap)
                            n_idx = m * P
                            nc.gpsimd.dma_gather(
                                out_ap=xgw[:].rearrange(
                                    "p (k w) -> p k w", w=RW)[:, :m, :],
                                in_ap=src,
                                idxs_ap=idxb[:, ioff:ioff + n_idx // 16],
                                num_idxs=n_idx,
                                num_idxs_reg=n_idx,
                                elem_size=RW,
                                single_packet=singlepkt,
                            )
                            widx += 1
                            ioff += n_idx // 16
                        xg3 = None
                    else:
                        xg = gpool.tile([P, K * RW], FT, tag="xg")
                        xg3 = xg[:].rearrange("p (k w) -> p k w", w=RW)
                        for (k0, k1) in plan[i]:
                            m = k1 - k0
                            if mode == "gnoreg":
                                # timing probe: one base for all windows
                                reg = pool_regs[0]
                                if widx == 0:
                                    nc.reg_load(reg, meta[0:1, 0:1])
                            else:
                                grp, off = divmod(widx, GRP)
                                bank = (grp % 2) * GRP
                                if off == 0:
                                    n = min(GRP, n_windows - widx)
                                    nc.reg_load(
                                        pool_regs[bank:bank + n],
                                        meta[0:1, widx:widx + n])
                                reg = pool_regs[bank + off]
                            src = bass.AP(src_rows.tensor,
                                          bass.RuntimeValue(reg), src_rows.ap)
                            n_idx = m * P
                            nc.gpsimd.dma_gather(
                                out_ap=xg3[:, k0:k1, :],
                                in_ap=src,
                                idxs_ap=idxb[:, ioff:ioff + n_idx // 16],
                                num_idxs=n_idx,
                                num_idxs_reg=n_idx,
                                elem_size=RW,
                                single_packet=singlepkt,
                            )
                            widx += 1
                            ioff += n_idx // 16

                    own = own_all[:, i, :]
                    mn = mn_all[:, i, :]

                    if mode in ("gather", "gsep", "gnoreg"):
                        if last:
                            z = spool.tile([P, S], FT, tag="z")
                            nc.vector.memset(z[:], 0.0)
                            nc.sync.dma_start(out=t_out[rows, :], in_=z[:])
                        else:
                            z = spool.tile([P, PACKW], FT, tag="z")
                            nc.vector.memset(z[:], 0.0)
                            nc.sync.dma_start(out=shards[s][rows, :PACKW],
                                              in_=z[:])
                        continue

                    # scores
                    sa_b = spool.tile([P, H], FT, tag="sa_b")
                    nc.vector.tensor_add(out=sa_b[:],
                                         in0=own[:, OFF_SB + H:OFF_SB + 2 * H],
                                         in1=bb[:])
                    e_hk_t = spool.tile([P, H, K], FT, tag="e_hk")
                    e_hk = e_hk_t[:][:, :, :kti]
                    sb_slot = xg3[:, :kti, OFF_SB:OFF_SB + H].rearrange(
                        "p k h -> p h k")
                    sa_b_bc = sa_b[:].rearrange(
                        "p (h o) -> p h o", o=1).to_broadcast([P, H, kti])
                    nc.vector.tensor_add(out=e_hk, in0=sb_slot, in1=sa_b_bc)
                    nc.scalar.activation(
                        out=e_hk, in_=e_hk,
                        func=mybir.ActivationFunctionType.Lrelu, alpha=ALPHA)
                    mn_b = mn[:, :kti].rearrange(
                        "p (o k) -> p o k", o=1).to_broadcast([P, H, kti])
                    nc.vector.tensor_add(out=e_hk, in0=e_hk, in1=mn_b)
                    Dn = spool.tile([P, H], FT, tag="Dn")
                    for h in range(H):
                        nc.scalar.activation(
                            out=e_hk[:, h, :], in_=e_hk[:, h, :],
                            func=mybir.ActivationFunctionType.Exp,
                            accum_out=Dn[:, h:h + 1])
                    e_self = spool.tile([P, H], FT, tag="e_self")
                    nc.vector.tensor_add(out=e_self[:], in0=sa_b[:],
                                         in1=own[:, OFF_SB:OFF_SB + H])
                    nc.scalar.activation(
                        out=e_self[:], in_=e_self[:],
                        func=mybir.ActivationFunctionType.Lrelu, alpha=ALPHA)
                    nc.scalar.activation(
                        out=e_self[:], in_=e_self[:],
                        func=mybir.ActivationFunctionType.Exp)
                    r4 = spool.tile([P, H], FT, tag="r4")
                    nc.vector.tensor_add(out=Dn[:], in0=Dn[:], in1=e_self[:])
                    nc.vector.reciprocal(out=r4[:], in_=Dn[:])
                    nc.vector.tensor_scalar_mul(out=r4[:], in0=r4[:],
                                                scalar1=1.0 / H)
                    p_kh_t = spool.tile([P, K, H], FT, tag="p_kh")
                    p_kh = p_kh_t[:][:, :kti, :]
                    e_as_kh = e_hk.rearrange("p h k -> p k h")
                    r4_b = r4[:].rearrange(
                        "p (o h) -> p o h", o=1).to_broadcast([P, kti, H])
                    nc.vector.tensor_mul(out=p_kh, in0=e_as_kh, in1=r4_b)
                    q_t = spool.tile([P, K], FT, tag="q")
                    q = q_t[:][:, :kti]
                    nc.vector.tensor_reduce(out=q, in_=p_kh,
                                            axis=mybir.AxisListType.X,
                                            op=mybir.AluOpType.add)
                    q0 = spool.tile([P, 1], FT, tag="q0")
                    es_r = spool.tile([P, H], FT, tag="es_r")
                    nc.vector.scalar_tensor_tensor(
                        out=es_r[:], in0=e_self[:], scalar=1.0, in1=r4[:],
                        op0=mybir.AluOpType.mult, op1=mybir.AluOpType.mult,
                        accum_out=q0[:])

                    # x payload views (bf16 mode reads bf16 in0 directly)
                    if BF16:
                        own_x = own[:, :XW].bitcast(BT)
                        def xg_x(k):
                            return xg3[:, k, :XW].bitcast(BT)
                    else:
                        own_x = own[:, :XW]
                        def xg_x(k):
                            return xg3[:, k, :XW]

                    # weighted sum: 4 interleaved FMA chains (DVE ILP)
                    CT = mybir.dt.float16 if chain16 else FT
                    acc = spool.tile([P, S], FT, tag="acc")
                    accs = [spool.tile([P, S], CT, tag=f"acc{j}",
                                       name=f"acc{j}") for j in range(4)]
                    nc.vector.tensor_scalar(
                        out=accs[0][:], in0=own_x, scalar1=q0[:, 0:1],
                        scalar2=None, op0=mybir.AluOpType.mult)
                    if kti >= 4:
                        for j in range(1, 4):
                            nc.vector.tensor_scalar(
                                out=accs[j][:], in0=xg_x(j),
                                scalar1=q[:, j:j + 1], scalar2=None,
                                op0=mybir.AluOpType.mult)
                        for k in range(4, kti):
                            a = accs[k % 4]
                            nc.vector.scalar_tensor_tensor(
                                out=a[:], in0=xg_x(k),
                                scalar=q[:, k:k + 1], in1=a[:],
                                op0=mybir.AluOpType.mult,
                                op1=mybir.AluOpType.add)
                        nc.vector.scalar_tensor_tensor(
                            out=accs[1][:], in0=xg_x(0),
                            scalar=q[:, 0:1], in1=accs[1][:],
                            op0=mybir.AluOpType.mult, op1=mybir.AluOpType.add)
                        nc.vector.tensor_add(out=accs[2][:], in0=accs[2][:],
                                             in1=accs[3][:])
                        nc.vector.tensor_add(out=accs[0][:], in0=accs[0][:],
                                             in1=accs[1][:])
                        nc.vector.tensor_add(out=acc[:], in0=accs[0][:],
                                             in1=accs[2][:])
                    else:
                        # rare tiny-K tiles: serial chain, last op lands in acc
                        for k in range(kti):
                            dst = acc if k == kti - 1 else accs[0]
                            nc.vector.scalar_tensor_tensor(
                                out=dst[:], in0=xg_x(k),
                                scalar=q[:, k:k + 1], in1=accs[0][:],
                                op0=mybir.AluOpType.mult,
                                op1=mybir.AluOpType.add)

                    if last:
                        outt = spool.tile([P, S], FT, tag="outt")
                        nc.scalar.activation(
                            out=outt[:], in_=acc[:],
                            func=mybir.ActivationFunctionType.Relu)
                        nc.sync.dma_start(out=t_out[rows, :], in_=outt[:])
                    else:
                        outf = spool.tile([P, S], FT, tag="outf")
                        nc.scalar.activation(
                            out=outf[:], in_=acc[:],
                            func=mybir.ActivationFunctionType.Relu)
                        outt = spool.tile([P, PACKW], FT, tag="outt")
                        if BF16:
                            nc.vector.tensor_copy(
                                out=outt[:, :XW].bitcast(BT), in_=outf[:])
                        else:
                            nc.scalar.copy(out=outt[:, :XW], in_=outf[:])
                        oT_ps = pp.tile([P, S], FT, tag="oT")
                        nc.tensor.transpose(out=oT_ps[:], in_=outf[:],
                                            identity=ident[:])
                        oT = spool.tile([S, P], FT, tag="oTs")
                        nc.vector.tensor_copy(out=oT[:], in_=oT_ps[:])
                        tail_ps = pp.tile([P, 2 * H], FT, tag="tail")
                        nc.tensor.matmul(out=tail_ps[:], lhsT=oT[:],
                                         rhs=wab[:], start=True, stop=True)
                        nc.vector.tensor_copy(out=outt[:, XW:PACKW],
                                              in_=tail_ps[:])
                        nc.sync.dma_start(
                            out=shards[s][rows, :PACKW], in_=outt[:])

    nc.compile()
    return nc


def make_runner(nc, in_maps):
    """Build a reusable jitted runner (mirrors bass2jax.run_bass_via_pjrt
    multi-core path, without output donation) + device-resident inputs.
    Returns (run_fn, split_fn). run_fn() executes and blocks; returns raw
    jax output arrays. split_fn(outs) -> per-core dicts."""
    import jax
    from jax.sharding import Mesh, NamedSharding, PartitionSpec
    from jax.experimental.shard_map import shard_map
    from concourse import bass2jax
    from concourse.bass2jax import _bass_exec_p, partition_id_tensor
    import concourse.mybir as mb

    bass2jax.install_neuronx_cc_hook()
    n_cores = len(in_maps)
    partition_name = nc.partition_id_tensor.name if nc.partition_id_tensor else None
    in_names, out_names, out_avals = [], [], []
    for alloc in nc.m.functions[0].allocations:
        if not isinstance(mb.MemoryLocationSet, type) or not isinstance(alloc, mb.MemoryLocationSet):
            continue
        name = alloc.memorylocations[0].name
        if alloc.kind == "ExternalInput":
            if name != partition_name:
                in_names.append(name)
        elif alloc.kind == "ExternalOutput":
            out_names.append(name)
            out_avals.append(jax.core.ShapedArray(tuple(alloc.tensor_shape),
                                                  mb.dt.np(alloc.dtype)))
    n_params = len(in_names)
    all_in_names = list(in_names)
    if partition_name is not None:
        all_in_names.append(partition_name)

    def _body(*args):
        operands = list(args)
        if partition_name is not None:
            operands.append(partition_id_tensor())
        outs = _bass_exec_p.bind(
            *operands,
            out_avals=tuple(out_avals),
            in_names=tuple(all_in_names),
            out_names=tuple(out_names),
            lowering_input_output_aliases=(),
            sim_require_finite=True,
            sim_require_nnan=True,
            nc=nc,
        )
        return tuple(outs)

    devices = jax.devices()[:n_cores]
    mesh = Mesh(np.asarray(devices), ("core",))
    sharded = jax.jit(shard_map(_body, mesh=mesh,
                                in_specs=(PartitionSpec("core"),) * n_params,
                                out_specs=(PartitionSpec("core"),) * len(out_names),
                                check_rep=False), keep_unused=True)
    concat_in = [np.concatenate([np.asarray(in_maps[c][nm])
                                 for c in range(n_cores)], axis=0)
                 for nm in in_names]
    # Pre-shard inputs across the cores so each run() dispatches with zero
    # input movement (an unsharded device_put re-scatters every call).
    shard_spec = NamedSharding(mesh, PartitionSpec("core"))
    dev_in = [jax.device_put(a, shard_spec) for a in concat_in]
    for a in dev_in:
        a.block_until_ready()

    def run_fn():
        outs = sharded(*dev_in)
        for o in outs:
            o.block_until_ready()
        return outs

    def split_fn(outs):
        res = [dict() for _ in range(n_cores)]
        for o, nm in zip(outs, out_names):
            o = np.asarray(o)
            per = o.shape[0] // n_cores
            for c in range(n_cores):
                res[c][nm] = o[c * per:(c + 1) * per]
        return res

    return run_fn, split_fn


_CACHE = {}


def _get_module(steps, g):
    tpc = int(os.environ.get("DQA_DEBUG_TPC", TPC))
    key = (steps, tpc)
    if key not in _CACHE:
        _CACHE[key] = _build(steps, g["plan"], g["n_windows"], tpc,
                             kt=g["kt"], idx_cols=g["idx_cols_total"])
    return _CACHE[key]


def _finalize(out_rows, g):
    """Un-permute device row order back to original node order."""
    oor = g["old_of_row"]
    sel = oor < N
    final = np.empty((N, S), np.float32)
    final[oor[sel]] = out_rows[sel]
    return final


def _make_in_maps(inputs, g):
    x = np.asarray(inputs["x"], np.float32)
    W = np.asarray(inputs["W"], np.float32)
    b = np.asarray(inputs["b"], np.float32)
    wa, wb = W[:, :S], W[:, S:]
    x_pad = np.zeros((NPAD, S), np.float32)
    x_pad[:N] = x
    x_row = x_pad[g["old_of_row"]]       # device row order
    sb0 = x_row @ wb.T
    sa0 = x_row @ wa.T
    table0 = _pack_rows(x_row, sb0, sa0)
    wab = np.concatenate([wb.T, wa.T], axis=1).astype(np.float32)
    bb = np.tile(b, (P, 1)).astype(np.float32)
    ident = np.eye(P, dtype=np.float32)
    in_maps = []
    for c in range(NCORES):
        rows = slice(c * SHARD, (c + 1) * SHARD)
        in_maps.append({
            "table0": table0,
            "own0": np.ascontiguousarray(table0[rows]),
            "idx_blob": g["idx_blobs"][c],
            "meta": g["meta_blobs"][c],
            "maskneg": np.ascontiguousarray(g["maskneg"][rows]),
            "wab": wab,
            "bb": bb,
            "ident": ident,
        })
    return in_maps


# ------------------------------------------------------------------- kernel
def kernel(x, W, b, neighbors, mask, propagate_count):
    x = np.ascontiguousarray(np.asarray(x, np.float32))
    W = np.asarray(W, np.float32)
    b = np.asarray(b, np.float32)
    steps = int(propagate_count)
    if steps <= 0:
        return x.copy()

    g = _prep_graph(neighbors, mask)
    nc = _get_module(steps, g)

    in_maps = _make_in_maps({"x": x, "W": W, "b": b}, g)
    res = run_bass_kernel_spmd(nc, in_maps, list(range(NCORES)))
    out = np.concatenate([res.results[c]["out_shard"] for c in range(NCORES)],
                         axis=0)
    return np.ascontiguousarray(_finalize(out, g))


if __name__ == "__main__":
    import jax
    sys.path.insert(0, os.path.dirname(os.path.abspath(__file__)))
    import reference
    with jax.default_device(jax.devices("cpu")[0]):
        inputs = reference.setup_inputs()
        inputs = {k: (np.asarray(v) if hasattr(v, "shape") else v)
                  for k, v in inputs.items()}
        expected = np.asarray(reference.reference(**inputs))
    got = kernel(**inputs)
    rel = np.linalg.norm(got - expected) / np.linalg.norm(expected)
    print(f"Relative error: {rel:.3e}")


# revision 48
# speedup vs baseline: 1.0045x; 1.0045x over previous
"""Trainium2 Bass kernel for nn_DQA_graph (GNN message passing, DQA attention).

Strategy (data-parallel over nodes, 8 cores):
  - Nodes padded to 50176 = 8 cores x 49 tiles x 128 rows; core c owns rows
    [c*6272, (c+1)*6272).
  - Node states live in a packed DRAM table whose rows hold [x | sb | sa]
    where sa/sb are the per-head DQA score contributions (h @ wa.T, h @ wb.T).
    The neighbor gather fetches x AND sb in one row read.
      * f32 mode  (DQA_BF16=0): rows of 192 f32 (768B): x f32[128] | sb | sa
      * bf16 mode (DQA_BF16=1): rows of 128 f32 (512B): x bf16[128] packed in
        64 f32 slots | sb f32[4] | sa f32[4] | pad
  - Step 0 reads a HOST-precomputed replicated table (no pack phase and no
    step-0 AllGather on device); each step s>=1 gathers from an AllGather of
    the packed rows produced by step s-1.
  - Per-node neighbor lists are pre-sorted ascending (host), so the K=32
    gather columns of a 128-node tile are order statistics; greedy grouping
    packs columns into windows whose index span fits dma_gather's int16
    range, with the window base supplied at runtime per (core, tile, window)
    from a metadata tensor (the program is SPMD-uniform across cores).
  - The gather is k-major: gathered row (k*128 + t) lands at partition t,
    chunk k -> the xg tile is directly [node t, slot k, row] with no
    transpose.
  - Scores/softmax run on ACT+DVE entirely in [t, *] layout; the weighted
    sum is 4 interleaved chains of scalar_tensor_tensor FMAs (per-partition
    scalar, fp16 accumulators) for DVE ILP.
  - Window base registers are bulk-loaded 8 at a time (double-banked) to
    amortize the ~0.8us/reg_load Pool-sequencer cost.
  - Perf notes (measured): the kernel is gather-bound; the dma_gather is
    HBM-latency-bound per descriptor (~140ns/row across 16 SDMA engines,
    ~53GB/s effective), so descriptor COUNT (one per neighbor row) is what
    matters, not row width. Runner pre-shards device inputs (NamedSharding)
    so each dispatch moves no input bytes.
"""
import os
import sys

sys.path.insert(0, "/opt/trn_rl_repo")
import numpy as np

import concourse.bacc as bacc
import concourse.bass as bass
import concourse.mybir as mybir
from concourse.bass_utils import run_bass_kernel_spmd
from concourse.tile import TileContext

# problem constants (hardcoded per harness contract)
N, K, S, H = 50000, 32, 128, 4
NCORES = 8
P = 128
TPC = 49                      # tiles per core
NPAD = NCORES * TPC * P       # 50176
SHARD = TPC * P               # 6272 rows per core
BF16 = bool(int(os.environ.get("DQA_BF16", "1")))
if BF16:
    RW = 128                  # packed row width (f32 slots) = 512B
    XW = 64                   # f32 slots holding the (bf16) x payload
else:
    RW = 192                  # 768B rows
    XW = 128
OFF_SB, OFF_SA = XW, XW + H
PACKW = XW + 2 * H            # meaningful prefix of a packed row
MAXW = 32768                  # int16 index window (rows)
MAXM = int(os.environ.get("DQA_MAXM", "16"))  # max columns per gather call
SINGLEPKT = MAXM <= 8
CHAIN16 = bool(int(os.environ.get("DQA_CHAIN16", "1")))  # fp16 FMA chains
NEG = -50.0
ALPHA = 0.01                  # leaky relu slope
FT = mybir.dt.float32
BT = mybir.dt.bfloat16


def _to_bf16_bits(a):
    """f32 ndarray -> uint16 bf16 bits (round to nearest even)."""
    v = a.astype(np.float32).view(np.uint32)
    r = (v + 0x7FFF + ((v >> 16) & 1)) >> 16
    return r.astype(np.uint16)


def _pack_rows(x, sb, sa):
    """Pack [n,S] f32 x (+[n,H] sb, sa) into [n, RW] f32-viewed rows."""
    n = x.shape[0]
    out = np.zeros((n, RW), np.float32)
    if BF16:
        bits = _to_bf16_bits(x)                      # [n, S] uint16
        out[:, :XW] = bits.view(np.uint32).view(np.float32)
    else:
        out[:, :XW] = x
    out[:, OFF_SB:OFF_SB + H] = sb
    out[:, OFF_SA:OFF_SA + H] = sa
    return out


# ----------------------------------------------------------------- host prep
def _prep_graph(neighbors, mask, maxm=None):
    """Valid-compacted, v-sorted gather plan.

    Nodes are permuted by valid-neighbor count v so that each tile position i
    draws its 8 cores' tiles from one contiguous v-sorted block of 1024 nodes
    -> every core shares the same per-tile column count K_i = max v in block
    (the SPMD program needs shared loop bounds). Each node's VALID neighbors
    are compacted to the front (sorted ascending for windowing); columns
    k >= v_t are padded with the node's last valid neighbor (masked -50).
    Invalid slots beyond K_i are simply never gathered (~25% fewer
    descriptors; the gather is HBM-latency-bound per descriptor).

    Returns dict with per-core input arrays, the window plan, per-tile K_i,
    and the node permutation (kernel() un-permutes the output)."""
    if maxm is None:
        maxm = MAXM
    nbr = np.asarray(neighbors, np.int64)
    msk = np.asarray(mask, bool)

    # padded node table: pads have v=0
    v = np.zeros(NPAD, np.int64)
    v[:N] = msk.sum(axis=1)
    perm = np.argsort(v, kind="stable")          # v-sorted rank s -> old node

    # table row id r (core-major: r = c*SHARD + i*P + t) for sorted rank s
    # with i = s // 1024, c = (s // 128) % 8, t = s % 128 — so tile position
    # i on every core draws from the same v-sorted block of 1024 nodes.
    s_arr = np.arange(NPAD)
    i_arr, c_arr, t_arr = s_arr // (NCORES * P), (s_arr // P) % NCORES, s_arr % P
    r_of_s = c_arr * SHARD + i_arr * P + t_arr
    old_of_row = np.empty(NPAD, np.int64)
    old_of_row[r_of_s] = perm                     # row r -> old node id
    row_of_old = np.empty(NPAD, np.int64)
    row_of_old[old_of_row] = np.arange(NPAD)      # old node id -> row r

    v_row = v[old_of_row]
    # per row: compacted, remapped, ascending-sorted valid neighbors
    nbr_c = np.zeros((NPAD, K), np.int64)
    # and full original-style sorted-32 lists (fallback mode)
    nbr_f = np.zeros((NPAD, K), np.int64)
    msk_f = np.zeros((NPAD, K), bool)
    for r in range(NPAD):
        o = old_of_row[r]
        if o < N:
            vals_all = row_of_old[nbr[o]]
            order = np.argsort(vals_all, kind="stable")
            nbr_f[r] = vals_all[order]
            msk_f[r] = msk[o][order]
            if v_row[r] > 0:
                vals = np.sort(vals_all[msk[o]])
                nbr_c[r, :v_row[r]] = vals
                nbr_c[r, v_row[r]:] = vals[-1]    # provisional pad
            else:
                nbr_c[r, :] = r
        else:
            nbr_c[r, :] = r                       # pad node (masked anyway)
            nbr_f[r, :] = r
    valid_col = np.arange(K)[None, :] < v_row[:, None]   # [NPAD, K]
    # pad slots are masked (-50) so their value only matters for window
    # spans: use the tile-column median of valid entries to keep columns
    # tight (last-valid padding puts a node's max in mid-quantile columns)
    for blk in range(NPAD // P):
        rows_b = slice(blk * P, (blk + 1) * P)
        for k in range(K):
            vc = valid_col[rows_b, k]
            if vc.all():
                continue
            if vc.any():
                med = int(np.median(nbr_c[rows_b, k][vc]))
            else:
                med = blk * P                     # all-pad column (v=0 tile)
            col = nbr_c[rows_b, k]
            col[~vc] = med
            nbr_c[rows_b, k] = col

    # per-tile-position column count, shared across cores (>=1)
    KT = [max(1, int(v[perm[i * NCORES * P:(i + 1) * NCORES * P]].max()))
          for i in range(TPC)]

    cols_c = nbr_c.reshape(NCORES, TPC, P, K).transpose(0, 1, 3, 2)
    cols_f = nbr_f.reshape(NCORES, TPC, P, K).transpose(0, 1, 3, 2)
    span_c = cols_c.max(axis=3) - cols_c.min(axis=3)  # [NCORES, TPC, K]

    # per tile position: compact unless any core has a single-column span
    # overflow within the compact K range -> fall back to the full-32 layout
    cols = cols_f.copy()
    maskneg = np.where(msk_f, 0.0, NEG).astype(np.float32)
    mask_rows = maskneg.reshape(NCORES, TPC, P, K)
    modes = []
    for i in range(TPC):
        if (span_c[:, i, :KT[i]] < MAXW).all():
            cols[:, i] = cols_c[:, i]
            mask_rows[:, i] = np.where(
                valid_col.reshape(NCORES, TPC, P, K)[:, i], 0.0, NEG)
            modes.append("compact")
        else:
            KT[i] = K
            modes.append("full")
    maskneg = mask_rows.reshape(NPAD, K).astype(np.float32)
    cmin = cols.min(axis=3)
    cmax = cols.max(axis=3)

    plan = []  # per tile position: list of (k0, k1), k1 <= KT[i]
    for i in range(TPC):
        wins = []
        k0 = 0
        while k0 < KT[i]:
            lo = cmin[:, i, k0].copy()
            hi = cmax[:, i, k0].copy()
            assert (hi - lo < MAXW).all(), "single column exceeds window"
            k1 = k0 + 1
            while k1 < KT[i] and k1 - k0 < maxm:
                nlo = np.minimum(lo, cmin[:, i, k1])
                nhi = np.maximum(hi, cmax[:, i, k1])
                if (nhi - nlo >= MAXW).any():
                    break
                lo, hi = nlo, nhi
                k1 += 1
            wins.append((k0, k1))
            k0 = k1
        plan.append(wins)

    # per-core blobs (variable total idx columns, shared shape across cores)
    idx_cols_total = sum((k1 - k0) * P for p in plan for (k0, k1) in p) // 16
    idx_blobs, meta_blobs = [], []
    for c in range(NCORES):
        idx_blob = np.zeros((16, idx_cols_total), np.int16)
        bases = []
        off = 0
        for i in range(TPC):
            tile_cols = cols[c, i]  # [K, P]
            for (k0, k1) in plan[i]:
                base = int(tile_cols[k0:k1].min())
                bases.append(base * RW)
                rel = (tile_cols[k0:k1] - base).astype(np.int64)  # [m, P]
                assert rel.min() >= 0 and rel.max() < MAXW
                flat = rel.reshape(-1).astype(np.int16)  # k-major
                m16 = flat.shape[0] // 16
                idx_blob[:, off:off + m16] = flat.reshape(m16, 16).T
                off += m16
        assert off == idx_cols_total
        idx_blobs.append(np.tile(idx_blob, (8, 1)))  # replicate to 128 parts
        meta_blobs.append(np.asarray(bases, np.int32).reshape(1, -1))

    return {
        "plan": plan,
        "kt": KT,
        "modes": modes,
        "idx_blobs": idx_blobs,
        "meta_blobs": meta_blobs,
        "maskneg": maskneg,          # already in row order
        "old_of_row": old_of_row,    # row r -> original node id
        "n_windows": len(meta_blobs[0][0]),
        "idx_cols_total": idx_cols_total,
    }


# ------------------------------------------------------------- device build
def _build(steps, plan, n_windows, tpc=TPC, mode="full", reps=1,
           chain16=None, singlepkt=None, gbufs=4, kt=None, idx_cols=None):
    """Build the SPMD Bacc module for `steps` propagation steps.

    mode: "full" | "gather" (skip per-tile compute) | "compute" (skip
    gathers, memset xg instead).
    reps: replicate the whole computation inside the NEFF (timing use:
    (wall_R - wall_1)/(R-1) cancels dispatch overhead)."""
    if chain16 is None:
        chain16 = CHAIN16
    if singlepkt is None:
        singlepkt = max(k1 - k0 for p in plan for (k0, k1) in p) <= 8
    if kt is None:
        kt = [K] * tpc
    if idx_cols is None:
        idx_cols = SHARD * K // 16
    nc = bacc.Bacc()
    t_tab0 = nc.dram_tensor("table0", [NPAD, RW], FT, kind="ExternalInput")
    t_own0 = nc.dram_tensor("own0", [SHARD, RW], FT, kind="ExternalInput")
    t_idx = nc.dram_tensor("idx_blob", [128, idx_cols], mybir.dt.int16,
                           kind="ExternalInput")
    t_meta = nc.dram_tensor("meta", [1, n_windows], mybir.dt.int32,
                            kind="ExternalInput")
    t_mn = nc.dram_tensor("maskneg", [SHARD, K], FT, kind="ExternalInput")
    t_wab = nc.dram_tensor("wab", [S, 2 * H], FT, kind="ExternalInput")
    t_bb = nc.dram_tensor("bb", [P, H], FT, kind="ExternalInput")
    t_ident = nc.dram_tensor("ident", [P, P], FT, kind="ExternalInput")
    t_out = nc.dram_tensor("out_shard", [SHARD, S], FT, kind="ExternalOutput")

    shards = [nc.dram_tensor(f"shard{s}", [SHARD, RW], FT)
              for s in range(1, steps)]
    tables = [nc.dram_tensor(f"table{s}", [NPAD, RW], FT, addr_space="Shared")
              for s in range(1, steps)]

    pool_regs = [list(nc.alloc_registers(f"gbase{j}",
                                         engines=[mybir.EngineType.Pool]))[0]
                 for j in range(16)]
    GRP = 8  # window bases bulk-loaded per reg_load, double-banked

    for rep in range(reps):
      for s in range(steps):
        if s > 0:
            # AllGather shard_{s} -> table_{s}
            with nc.Block() as block, \
                 nc.semaphore(f"ccs{rep}_{s}") as cc_sem:
                @block.gpsimd
                def _(gpsimd, s=s, cc_sem=cc_sem):
                    gpsimd.collective_compute(
                        "AllGather", mybir.AluOpType.bypass,
                        replica_groups=[list(range(NCORES))],
                        ins=[shards[s - 1][:]], outs=[tables[s - 1][:]],
                    ).then_inc(cc_sem, 1)
                    gpsimd.wait_ge(cc_sem, 1)

        last = (s == steps - 1)
        with TileContext(nc) as tc:
            with tc.tile_pool(name=f"g{rep}_{s}", bufs=gbufs) as gpool, \
                 tc.tile_pool(name=f"sm{rep}_{s}", bufs=3) as spool, \
                 tc.tile_pool(name=f"c{rep}_{s}", bufs=1) as cpool, \
                 tc.tile_pool(name=f"ps{rep}_{s}", bufs=2, space="PSUM") as pp:
                idxb = cpool.tile([128, idx_cols], mybir.dt.int16)
                meta = cpool.tile([1, n_windows], mybir.dt.int32)
                wab = cpool.tile([S, 2 * H], FT)
                bb = cpool.tile([P, H], FT)
                ident = cpool.tile([P, P], FT)
                nc.sync.dma_start(out=idxb[:], in_=t_idx[:])
                nc.sync.dma_start(out=meta[:], in_=t_meta[:])
                nc.sync.dma_start(out=wab[:], in_=t_wab[:])
                nc.sync.dma_start(out=bb[:], in_=t_bb[:])
                nc.sync.dma_start(out=ident[:], in_=t_ident[:])
                # bulk per-step loads: own rows + mask for all tiles
                own_src = t_own0 if s == 0 else shards[s - 1]
                own_all = cpool.tile([P, tpc, PACKW], FT)
                nc.sync.dma_start(
                    out=own_all[:],
                    in_=own_src[:tpc * P, :PACKW].rearrange(
                        "(i p) w -> p i w", p=P))
                mn_all = cpool.tile([P, tpc, K], FT)
                nc.sync.dma_start(
                    out=mn_all[:],
                    in_=t_mn[:tpc * P, :].rearrange("(i p) k -> p i k", p=P))

                src_rows = (t_tab0 if s == 0 else tables[s - 1])[:]
                widx = 0
                ioff = 0
                for i in range(tpc):
                    rows = slice(i * P, (i + 1) * P)
                    kti = kt[i]
                    if mode == "compute":
                        if i == 0:
                            xg_shared = cpool.tile([P, K * RW], FT,
                                                   name=f"xgsh{rep}_{s}")
                            nc.vector.memset(xg_shared[:], 0.01)
                        xg3 = xg_shared[:].rearrange("p (k w) -> p k w", w=RW)
                        widx += len(plan[i])
                        ioff += K * P // 16
                    elif mode == "gsep":
                        # timing probe: each window gathers into its own tile
                        if i == 0 and rep == 0 and s == 0:
                            pass
                        for (k0, k1) in plan[i]:
                            m = k1 - k0
                            xgw = gpool.tile([P, MAXM * RW], FT, tag="xgw")
                            reg = pool_regs[widx % len(pool_regs)]
                            nc.reg_load(reg, meta[0:1, widx:widx + 1])
                            src = bass.AP(src_rows.tensor,
                                          bass.RuntimeValue(reg), src_rows.ap)
                            n_idx = m * P
                            nc.gpsimd.dma_gather(
                                out_ap=xgw[:].rearrange(
                                    "p (k w) -> p k w", w=RW)[:, :m, :],
                                in_ap=src,
                                idxs_ap=idxb[:, ioff:ioff + n_idx // 16],
                                num_idxs=n_idx,
                                num_idxs_reg=n_idx,
                                elem_size=RW,
                                single_packet=singlepkt,
                            )
                            widx += 1
                            ioff += n_idx // 16
                        xg3 = None
                    else:
                        xg = gpool.tile([P, K * RW], FT, tag="xg")
                        xg3 = xg[:].rearrange("p (k w) -> p k w", w=RW)
                        for (k0, k1) in plan[i]:
                            m = k1 - k0
                            if mode == "gnoreg":
                                # timing probe: one base for all windows
                                reg = pool_regs[0]
                                if widx == 0:
                                    nc.reg_load(reg, meta[0:1, 0:1])
                            else:
                                grp, off = divmod(widx, GRP)
                                bank = (grp % 2) * GRP
                                if off == 0:
                                    n = min(GRP, n_windows - widx)
                                    nc.reg_load(
                                        pool_regs[bank:bank + n],
                                        meta[0:1, widx:widx + n])
                                reg = pool_regs[bank + off]
                            src = bass.AP(src_rows.tensor,
                                          bass.RuntimeValue(reg), src_rows.ap)
                            n_idx = m * P
                            nc.gpsimd.dma_gather(
                                out_ap=xg3[:, k0:k1, :],
                                in_ap=src,
                                idxs_ap=idxb[:, ioff:ioff + n_idx // 16],
                                num_idxs=n_idx,
                                num_idxs_reg=n_idx,
                                elem_size=RW,
                                single_packet=singlepkt,
                            )
                            widx += 1
                            ioff += n_idx // 16

                    own = own_all[:, i, :]
                    mn = mn_all[:, i, :]

                    if mode in ("gather", "gsep", "gnoreg"):
                        if last:
                            z = spool.tile([P, S], FT, tag="z")
                            nc.vector.memset(z[:], 0.0)
                            nc.sync.dma_start(out=t_out[rows, :], in_=z[:])
                        else:
                            z = spool.tile([P, PACKW], FT, tag="z")
                            nc.vector.memset(z[:], 0.0)
                            nc.sync.dma_start(out=shards[s][rows, :PACKW],
                                              in_=z[:])
                        continue

                    # scores
                    sa_b = spool.tile([P, H], FT, tag="sa_b")
                    nc.vector.tensor_add(out=sa_b[:],
                                         in0=own[:, OFF_SB + H:OFF_SB + 2 * H],
                                         in1=bb[:])
                    e_hk_t = spool.tile([P, H, K], FT, tag="e_hk")
                    e_hk = e_hk_t[:][:, :, :kti]
                    sb_slot = xg3[:, :kti, OFF_SB:OFF_SB + H].rearrange(
                        "p k h -> p h k")
                    sa_b_bc = sa_b[:].rearrange(
                        "p (h o) -> p h o", o=1).to_broadcast([P, H, kti])
                    nc.vector.tensor_add(out=e_hk, in0=sb_slot, in1=sa_b_bc)
                    nc.scalar.activation(
                        out=e_hk, in_=e_hk,
                        func=mybir.ActivationFunctionType.Lrelu, alpha=ALPHA)
                    mn_b = mn[:, :kti].rearrange(
                        "p (o k) -> p o k", o=1).to_broadcast([P, H, kti])
                    nc.vector.tensor_add(out=e_hk, in0=e_hk, in1=mn_b)
                    Dn = spool.tile([P, H], FT, tag="Dn")
                    for h in range(H):
                        nc.scalar.activation(
                            out=e_hk[:, h, :], in_=e_hk[:, h, :],
                            func=mybir.ActivationFunctionType.Exp,
                            accum_out=Dn[:, h:h + 1])
                    e_self = spool.tile([P, H], FT, tag="e_self")
                    nc.vector.tensor_add(out=e_self[:], in0=sa_b[:],
                                         in1=own[:, OFF_SB:OFF_SB + H])
                    nc.scalar.activation(
                        out=e_self[:], in_=e_self[:],
                        func=mybir.ActivationFunctionType.Lrelu, alpha=ALPHA)
                    nc.scalar.activation(
                        out=e_self[:], in_=e_self[:],
                        func=mybir.ActivationFunctionType.Exp)
                    r4 = spool.tile([P, H], FT, tag="r4")
                    nc.vector.tensor_add(out=Dn[:], in0=Dn[:], in1=e_self[:])
                    nc.vector.reciprocal(out=r4[:], in_=Dn[:])
                    nc.vector.tensor_scalar_mul(out=r4[:], in0=r4[:],
                                                scalar1=1.0 / H)
                    p_kh_t = spool.tile([P, K, H], FT, tag="p_kh")
                    p_kh = p_kh_t[:][:, :kti, :]
                    e_as_kh = e_hk.rearrange("p h k -> p k h")
                    r4_b = r4[:].rearrange(
                        "p (o h) -> p o h", o=1).to_broadcast([P, kti, H])
                    nc.vector.tensor_mul(out=p_kh, in0=e_as_kh, in1=r4_b)
                    q_t = spool.tile([P, K], FT, tag="q")
                    q = q_t[:][:, :kti]
                    nc.vector.tensor_reduce(out=q, in_=p_kh,
                                            axis=mybir.AxisListType.X,
                                            op=mybir.AluOpType.add)
                    q0 = spool.tile([P, 1], FT, tag="q0")
                    es_r = spool.tile([P, H], FT, tag="es_r")
                    nc.vector.scalar_tensor_tensor(
                        out=es_r[:], in0=e_self[:], scalar=1.0, in1=r4[:],
                        op0=mybir.AluOpType.mult, op1=mybir.AluOpType.mult,
                        accum_out=q0[:])

                    # x payload views (bf16 mode reads bf16 in0 directly)
                    if BF16:
                        own_x = own[:, :XW].bitcast(BT)
                        def xg_x(k):
                            return xg3[:, k, :XW].bitcast(BT)
                    else:
                        own_x = own[:, :XW]
                        def xg_x(k):
                            return xg3[:, k, :XW]

                    # weighted sum: 4 interleaved FMA chains (DVE ILP)
                    CT = mybir.dt.float16 if chain16 else FT
                    acc = spool.tile([P, S], FT, tag="acc")
                    accs = [spool.tile([P, S], CT, tag=f"acc{j}",
                                       name=f"acc{j}") for j in range(4)]
                    nc.vector.tensor_scalar(
                        out=accs[0][:], in0=own_x, scalar1=q0[:, 0:1],
                        scalar2=None, op0=mybir.AluOpType.mult)
                    if kti >= 4:
                        for j in range(1, 4):
                            nc.vector.tensor_scalar(
                                out=accs[j][:], in0=xg_x(j),
                                scalar1=q[:, j:j + 1], scalar2=None,
                                op0=mybir.AluOpType.mult)
                        for k in range(4, kti):
                            a = accs[k % 4]
                            nc.vector.scalar_tensor_tensor(
                                out=a[:], in0=xg_x(k),
                                scalar=q[:, k:k + 1], in1=a[:],
                                op0=mybir.AluOpType.mult,
                                op1=mybir.AluOpType.add)
                        nc.vector.scalar_tensor_tensor(
                            out=accs[1][:], in0=xg_x(0),
                            scalar=q[:, 0:1], in1=accs[1][:],
                            op0=mybir.AluOpType.mult, op1=mybir.AluOpType.add)
                        nc.vector.tensor_add(out=accs[2][:], in0=accs[2][:],
                                             in1=accs[3][:])
                        nc.vector.tensor_add(out=accs[0][:], in0=accs[0][:],
                                             in1=accs[1][:])
                        nc.vector.tensor_add(out=acc[:], in0=accs[0][:],
                                             in1=accs[2][:])
                    else:
                        # rare tiny-K tiles: serial chain, last op lands in acc
                        for k in range(kti):
                            dst = acc if k == kti - 1 else accs[0]
                            nc.vector.scalar_tensor_tensor(
                                out=dst[:], in0=xg_x(k),
                                scalar=q[:, k:k + 1], in1=accs[0][:],
                                op0=mybir.AluOpType.mult,
                                op1=mybir.AluOpType.add)

                    if last:
                        outt = spool.tile([P, S], FT, tag="outt")
                        nc.scalar.activation(
                            out=outt[:], in_=acc[:],
                            func=mybir.ActivationFunctionType.Relu)
                        nc.sync.dma_start(out=t_out[rows, :], in_=outt[:])
                    else:
                        outf = spool.tile([P, S], FT, tag="outf")
                        nc.scalar.activation(
                            out=outf[:], in_=acc[:],
                            func=mybir.ActivationFunctionType.Relu)
                        outt = spool.tile([P, PACKW], FT, tag="outt")
                        if BF16:
                            nc.vector.tensor_copy(
                                out=outt[:, :XW].bitcast(BT), in_=outf[:])
                        else:
                            nc.scalar.copy(out=outt[:, :XW], in_=outf[:])
                        oT_ps = pp.tile([P, S], FT, tag="oT")
                        nc.tensor.transpose(out=oT_ps[:], in_=outf[:],
                                            identity=ident[:])
                        oT = spool.tile([S, P], FT, tag="oTs")
                        nc.vector.tensor_copy(out=oT[:], in_=oT_ps[:])
                        tail_ps = pp.tile([P, 2 * H], FT, tag="tail")
                        nc.tensor.matmul(out=tail_ps[:], lhsT=oT[:],
                                         rhs=wab[:], start=True, stop=True)
                        nc.vector.tensor_copy(out=outt[:, XW:PACKW],
                                              in_=tail_ps[:])
                        nc.sync.dma_start(
                            out=shards[s][rows, :PACKW], in_=outt[:])

    nc.compile()
    return nc


def make_runner(nc, in_maps):
    """Build a reusable jitted runner (mirrors bass2jax.run_bass_via_pjrt
    multi-core path, without output donation) + device-resident inputs.
    Returns (run_fn, split_fn). run_fn() executes and blocks; returns raw
    jax output arrays. split_fn(outs) -> per-core dicts."""
    import jax
    from jax.sharding import Mesh, NamedSharding, PartitionSpec
    from jax.experimental.shard_map import shard_map
    from concourse import bass2jax
    from concourse.bass2jax import _bass_exec_p, partition_id_tensor
    import concourse.mybir as mb

    bass2jax.install_neuronx_cc_hook()
    n_cores = len(in_maps)
    partition_name = nc.partition_id_tensor.name if nc.partition_id_tensor else None
    in_names, out_names, out_avals = [], [], []
    for alloc in nc.m.functions[0].allocations:
        if not isinstance(mb.MemoryLocationSet, type) or not isinstance(alloc, mb.MemoryLocationSet):
            continue
        name = alloc.memorylocations[0].name
        if alloc.kind == "ExternalInput":
            if name != partition_name:
                in_names.append(name)
        elif alloc.kind == "ExternalOutput":
            out_names.append(name)
            out_avals.append(jax.core.ShapedArray(tuple(alloc.tensor_shape),
                                                  mb.dt.np(alloc.dtype)))
    n_params = len(in_names)
    all_in_names = list(in_names)
    if partition_name is not None:
        all_in_names.append(partition_name)

    def _body(*args):
        operands = list(args)
        if partition_name is not None:
            operands.append(partition_id_tensor())
        outs = _bass_exec_p.bind(
            *operands,
            out_avals=tuple(out_avals),
            in_names=tuple(all_in_names),
            out_names=tuple(out_names),
            lowering_input_output_aliases=(),
            sim_require_finite=True,
            sim_require_nnan=True,
            nc=nc,
        )
        return tuple(outs)

    devices = jax.devices()[:n_cores]
    mesh = Mesh(np.asarray(devices), ("core",))
    sharded = jax.jit(shard_map(_body, mesh=mesh,
                                in_specs=(PartitionSpec("core"),) * n_params,
                                out_specs=(PartitionSpec("core"),) * len(out_names),
                                check_rep=False), keep_unused=True)
    concat_in = [np.concatenate([np.asarray(in_maps[c][nm])
                                 for c in range(n_cores)], axis=0)
                 for nm in in_names]
    # Pre-shard inputs across the cores so each run() dispatches with zero
    # input movement (an unsharded device_put re-scatters every call).
    shard_spec = NamedSharding(mesh, PartitionSpec("core"))
    dev_in = [jax.device_put(a, shard_spec) for a in concat_in]
    for a in dev_in:
        a.block_until_ready()

    def run_fn():
        outs = sharded(*dev_in)
        for o in outs:
            o.block_until_ready()
        return outs

    def split_fn(outs):
        res = [dict() for _ in range(n_cores)]
        for o, nm in zip(outs, out_names):
            o = np.asarray(o)
            per = o.shape[0] // n_cores
            for c in range(n_cores):
                res[c][nm] = o[c * per:(c + 1) * per]
        return res

    return run_fn, split_fn


_CACHE = {}


def _get_module(steps, g):
    tpc = int(os.environ.get("DQA_DEBUG_TPC", TPC))
    key = (steps, tpc)
    if key not in _CACHE:
        _CACHE[key] = _build(steps, g["plan"], g["n_windows"], tpc,
                             kt=g["kt"], idx_cols=g["idx_cols_total"])
    return _CACHE[key]


def _finalize(out_rows, g):
    """Un-permute device row order back to original node order."""
    oor = g["old_of_row"]
    sel = oor < N
    final = np.empty((N, S), np.float32)
    final[oor[sel]] = out_rows[sel]
    return final


def _make_in_maps(inputs, g):
    x = np.asarray(inputs["x"], np.float32)
    W = np.asarray(inputs["W"], np.float32)
    b = np.asarray(inputs["b"], np.float32)
    wa, wb = W[:, :S], W[:, S:]
    x_pad = np.zeros((NPAD, S), np.float32)
    x_pad[:N] = x
    x_row = x_pad[g["old_of_row"]]       # device row order
    sb0 = x_row @ wb.T
    sa0 = x_row @ wa.T
    table0 = _pack_rows(x_row, sb0, sa0)
    wab = np.concatenate([wb.T, wa.T], axis=1).astype(np.float32)
    bb = np.tile(b, (P, 1)).astype(np.float32)
    ident = np.eye(P, dtype=np.float32)
    in_maps = []
    for c in range(NCORES):
        rows = slice(c * SHARD, (c + 1) * SHARD)
        in_maps.append({
            "table0": table0,
            "own0": np.ascontiguousarray(table0[rows]),
            "idx_blob": g["idx_blobs"][c],
            "meta": g["meta_blobs"][c],
            "maskneg": np.ascontiguousarray(g["maskneg"][rows]),
            "wab": wab,
            "bb": bb,
            "ident": ident,
        })
    return in_maps


# ------------------------------------------------------------------- kernel
def kernel(x, W, b, neighbors, mask, propagate_count):
    x = np.ascontiguousarray(np.asarray(x, np.float32))
    W = np.asarray(W, np.float32)
    b = np.asarray(b, np.float32)
    steps = int(propagate_count)
    if steps <= 0:
        return x.copy()

    g = _prep_graph(neighbors, mask)
    nc = _get_module(steps, g)

    in_maps = _make_in_maps({"x": x, "W": W, "b": b}, g)
    res = run_bass_kernel_spmd(nc, in_maps, list(range(NCORES)))
    out = np.concatenate([res.results[c]["out_shard"] for c in range(NCORES)],
                         axis=0)
    return np.ascontiguousarray(_finalize(out, g))


if __name__ == "__main__":
    import jax
    sys.path.insert(0, os.path.dirname(os.path.abspath(__file__)))
    import reference
    with jax.default_device(jax.devices("cpu")[0]):
        inputs = reference.setup_inputs()
        inputs = {k: (np.asarray(v) if hasattr(v, "shape") else v)
                  for k, v in inputs.items()}
        expected = np.asarray(reference.reference(**inputs))
    got = kernel(**inputs)
    rel = np.linalg.norm(got - expected) / np.linalg.norm(expected)
    print(f"Relative error: {rel:.3e}")
